# revision 5
# baseline (speedup 1.0000x reference)
"""Trainium2 Bass kernel for the CNF reversible backward solve.

Math restructuring (exact, validated in fp64 against the jax reference):

The per-step recursion is tracked purely in H-space (H=256) via
Z = W1 z + bp(s), Yhat = l*(W1 y + be(s)):
    a_even = tanh(inv_l * Yhat)
    Z     += Mz @ a_even  (+ per-step bias delta)      (Mz = -h W1 W2)
    a_odd  = tanh(Z)
    Yhat'  = dby(s) + inv_l*q + Mz @ a_odd,   q = Yhat + l(l-1)*Z_post

Both states live in PSUM banks updated by matmuls only; biases ride in via
tiny rank-2 matmuls. The carry term q = Yhat + l(l-1)*Z_post is ONE vector
op (scalar_tensor_tensor) reading both PSUM banks post-z-update, injected
into the Y accumulation through a single fp32r identity matmul with the
inv_l factor baked into the identity's diagonal. Compared to the previous
revision this kills the (l-1)*Mz matmuls, the Z_pre anti-dependency, and
the compensated-bf16 hi/lo split (CAST + STT + 2 identity matmuls).

A ~3.5us dummy-matmul warmup burst at kernel start flips the PE HAM clock
gate from 1.2 GHz to 2.4 GHz before the main loop (the loop alone never
keeps the PE busy for a full 3.4us activity window, so without the burst
every matmul runs at half clock forever).

The device streams all activations a_e, a_o to DRAM; the D-space outputs
are exact fp64 host-side postprocessing:
    y_final = c_y y1 + sum_e gamma_e (W2 @ a_e) + c_b b2
    I_final = h (N sum(c) - sum_s c . a_even_s^2),   c = diag(W1 W2)

Sharding: data-parallel, B=256 -> 32 samples on each of 8 cores;
parameters replicated; gather + assembly on host.
"""

import numpy as np
import ml_dtypes
from contextlib import ExitStack

import concourse.bass as bass
import concourse.tile as tile
from concourse import bacc, mybir
from concourse.bass_utils import run_bass_kernel_spmd

# Problem constants (hardcoded per contract)
NCORES = 8
B, D, H = 256, 64, 256
NSTEP = 64
HSTEP = 1.0 / NSTEP
LCOUP = 0.999
INVL = 1.0 / LCOUP
BS = B // NCORES  # 32 samples per core
BSH = BS  # samples per chain (single chain per core)
NBLK = H // 128  # 2 h-blocks
FREE = NBLK * BSH  # 64: free size of H-space tiles, layout (blk, sample)
NEVAL = 2 * NSTEP  # 128
DMA_CHUNKS = 4
CSTEPS = NSTEP // DMA_CHUNKS  # steps per out-DMA chunk
CCOLS = CSTEPS * FREE
ACOLS = NSTEP * FREE  # columns in each activation stream

NWARM = 9  # dummy 512-col matmuls to warm the PE clock gate (~3.5us)

F32 = mybir.dt.float32
F32R = mybir.dt.float32r
BF16 = mybir.dt.bfloat16
BF16NP = ml_dtypes.bfloat16

SHARED_INPUTS = ["w1t", "mzt", "ib32", "dbz", "dby", "dbz0", "dby0", "ind", "indb"]


def _coefficients():
    """Exact fp64 scalar recursions for the output-extraction weights."""
    gamma = np.zeros(NEVAL)
    la = np.zeros(NEVAL)
    alpha_y = alpha_z = 1.0
    nu_y = nu_z = 0.0
    for s in range(NSTEP):
        la[2 * s] += -HSTEP
        nu_z += -HSTEP
        gamma *= INVL
        alpha_y *= INVL
        nu_y *= INVL
        gamma += (1.0 - INVL) * la
        alpha_y += (1.0 - INVL) * alpha_z
        nu_y += (1.0 - INVL) * nu_z
        gamma[2 * s + 1] += -INVL * HSTEP
        nu_y += -INVL * HSTEP
    return gamma, alpha_y, nu_y


def _host_tables(W1, b1, u1, W2, b2):
    """All precomputed tensors, fp64 internally."""
    W1 = W1.astype(np.float64)
    W2 = W2.astype(np.float64)
    b1 = b1.astype(np.float64)
    u1 = u1.astype(np.float64)
    b2 = b2.astype(np.float64)

    Mz = -HSTEP * (W1 @ W2)  # [H, H]
    W1b2 = W1 @ b2  # [H]
    l = LCOUP

    def be(s):
        return b1 + (1.0 - s * HSTEP) * u1

    def bp(s):  # beta_odd
        return b1 + (1.0 - (s + 1) * HSTEP) * u1 - (s + 1) * HSTEP * W1b2

    # mzt_pack[p, (k*NBLK+j)*128 + m] = Mz[128*j+m, 128*k+p]
    MzT = Mz.T
    mzt_pack = np.zeros((128, NBLK * NBLK * 128))
    for k in range(NBLK):
        for j in range(NBLK):
            mzt_pack[:, (k * NBLK + j) * 128 : (k * NBLK + j + 1) * 128] = MzT[
                128 * k : 128 * k + 128, 128 * j : 128 * j + 128
            ]

    # rank-2 bias tables: lhsT slice [2, 128] at cols 128*s
    dbz = np.zeros((2, NSTEP * 128))
    dby = np.zeros((2, NSTEP * 128))
    for s in range(NSTEP):
        dz = bp(s) if s == 0 else bp(s) - bp(s - 1)
        for k in range(NBLK):
            dbz[k, s * 128 : (s + 1) * 128] = dz[128 * k : 128 * k + 128]
    for s in range(NSTEP - 1):
        dh = -HSTEP * W1b2 + l * be(s + 1) - (l - 1.0) * bp(s) - be(s)
        for k in range(NBLK):
            dby[k, s * 128 : (s + 1) * 128] = dh[128 * k : 128 * k + 128]
    # col-block NSTEP-1 of dby = init bias l*be(0)
    ib = l * be(0)
    for k in range(NBLK):
        dby[k, (NSTEP - 1) * 128 : NSTEP * 128] = ib[128 * k : 128 * k + 128]

    ind = np.zeros((2, FREE))
    for k in range(NBLK):
        ind[k, k * BSH : (k + 1) * BSH] = 1.0

    dbz0 = dbz[:, 0:128].astype(np.float32)
    dby0 = dby[:, (NSTEP - 1) * 128 : NSTEP * 128].astype(np.float32)

    return dict(
        mzt=mzt_pack.astype(BF16NP),
        ib32=(INVL * np.eye(128)).astype(np.float32),
        dbz=dbz.astype(BF16NP),
        dby=dby.astype(BF16NP),
        dbz0=dbz0,
        dby0=dby0,
        ind=ind.astype(np.float32),
        indb=ind.astype(BF16NP),
        w1t=W1.T.astype(np.float32),
    )


def _build_kernel():
    """Build the Bass module (same program for every core)."""
    nc = bacc.Bacc("TRN2", target_bir_lowering=False, debug=False)

    y1t_d = nc.dram_tensor("y1t0", [D, BSH], F32, kind="ExternalInput").ap()
    y1tl_d = nc.dram_tensor("y1tl0", [D, BSH], F32, kind="ExternalInput").ap()
    w1t_d = nc.dram_tensor("w1t", [D, H], F32, kind="ExternalInput").ap()
    mzt_d = nc.dram_tensor("mzt", [128, NBLK * NBLK * 128], BF16, kind="ExternalInput").ap()
    ib32_d = nc.dram_tensor("ib32", [128, 128], F32R, kind="ExternalInput").ap()
    dbz_d = nc.dram_tensor("dbz", [2, NSTEP * 128], BF16, kind="ExternalInput").ap()
    dby_d = nc.dram_tensor("dby", [2, NSTEP * 128], BF16, kind="ExternalInput").ap()
    dbz0_d = nc.dram_tensor("dbz0", [2, 128], F32, kind="ExternalInput").ap()
    dby0_d = nc.dram_tensor("dby0", [2, 128], F32, kind="ExternalInput").ap()
    ind_d = nc.dram_tensor("ind", [2, FREE], F32, kind="ExternalInput").ap()
    indb_d = nc.dram_tensor("indb", [2, FREE], BF16, kind="ExternalInput").ap()

    ae_out_d = nc.dram_tensor("ae_out0", [128, ACOLS], BF16, kind="ExternalOutput").ap()
    ao_out_d = nc.dram_tensor("ao_out0", [128, ACOLS], BF16, kind="ExternalOutput").ap()

    with tile.TileContext(nc) as tc, ExitStack() as ctx:
        consts = ctx.enter_context(tc.tile_pool(name="consts", bufs=1))
        zpool = ctx.enter_context(tc.tile_pool(name="zps", bufs=1, space="PSUM"))
        ypool = ctx.enter_context(tc.tile_pool(name="yps", bufs=2, space="PSUM"))
        wpool = ctx.enter_context(tc.tile_pool(name="wps", bufs=1, space="PSUM"))
        ppool = ctx.enter_context(tc.tile_pool(name="ptmp", bufs=2))

        # --- prime the tanh activation table early (dep-free) ---
        warm = consts.tile([1, 8], F32, tag="warm")
        nc.vector.memset(warm[:], 0.0)
        nc.scalar.activation(warm[:], warm[:], mybir.ActivationFunctionType.Tanh)

        # --- PE clock-gate warmup: ~3.5us of dummy matmuls (dep-free) ---
        wsrc = consts.tile([128, 512], BF16, tag="wsrc", name="wsrc")
        nc.vector.memset(wsrc[:], 0.0)
        wps = wpool.tile([128, 512], F32, tag="wps", name="wps")
        for i in range(NWARM):
            nc.tensor.matmul(wps[:], wsrc[:, 0:128], wsrc[:], start=True, stop=True)

        # --- load constants (in consumption order) ---
        def cload(name, shape, dt, dram):
            t = consts.tile(shape, dt, tag=name, name=name)
            nc.sync.dma_start(t[:], dram)
            return t

        y1t = cload("y1t0", [D, BSH], F32, y1t_d)
        y1tl = cload("y1tl0", [D, BSH], F32, y1tl_d)
        w1t = cload("w1t", [D, H], F32, w1t_d)
        dbz0 = cload("dbz0", [2, 128], F32, dbz0_d)
        dby0 = cload("dby0", [2, 128], F32, dby0_d)
        ind = cload("ind", [2, FREE], F32, ind_d)
        mzt = cload("mzt", [128, NBLK * NBLK * 128], BF16, mzt_d)
        indb = cload("indb", [2, FREE], BF16, indb_d)
        dby = cload("dby", [2, NSTEP * 128], BF16, dby_d)
        dbz = cload("dbz", [2, NSTEP * 128], BF16, dbz_d)
        ib32 = cload("ib32", [128, 128], F32R, ib32_d)

        abuf_e = [
            consts.tile([128, CCOLS], BF16, tag=f"abe{c}", name=f"abe{c}")
            for c in range(DMA_CHUNKS)
        ]
        abuf_o = [
            consts.tile([128, CCOLS], BF16, tag=f"abo{c}", name=f"abo{c}")
            for c in range(DMA_CHUNKS)
        ]

        def mzt_blk(k, j):
            base = (k * NBLK + j) * 128
            return mzt[:, base : base + 128]

        # --- init: Z = W1 y1 + bp(0);  Yhat = l W1 y1 + l be(0) ---
        z_ps = zpool.tile([128, FREE], F32, tag="z", name="z")
        for j in range(NBLK):
            nc.tensor.matmul(
                z_ps[:, j * BSH : (j + 1) * BSH],
                w1t[:, 128 * j : 128 * j + 128],
                y1t[:],
                start=(j == 0),
                stop=False,
            )
        nc.tensor.matmul(z_ps[:], dbz0[:], ind[:], start=False, stop=True)

        y_cur = ypool.tile([128, FREE], F32, tag="y", name="yinit")
        for j in range(NBLK):
            nc.tensor.matmul(
                y_cur[:, j * BSH : (j + 1) * BSH],
                w1t[:, 128 * j : 128 * j + 128],
                y1tl[:],
                start=(j == 0),
                stop=False,
            )
        nc.tensor.matmul(y_cur[:], dby0[:], ind[:], start=False, stop=True)

        for s in range(NSTEP):
            last = s == NSTEP - 1
            chunk, cstep = divmod(s, CSTEPS)
            ecol = cstep * FREE

            # --- even eval: a_e = tanh(inv_l * Yhat) ---
            a_even = abuf_e[chunk][:, ecol : ecol + FREE]
            nc.scalar.activation(
                a_even[:], y_cur[:], mybir.ActivationFunctionType.Tanh,
                scale=INVL,
            )

            if not last:
                # SBUF copy of Yhat (DVE is otherwise idle during the even
                # ACT; the q STT below may read only one PSUM operand)
                yc_t = ppool.tile([128, FREE], F32, tag="yc", name=f"yc_{s}")
                nc.vector.tensor_copy(yc_t[:], y_cur[:])

            # --- Z += Mz @ a_even ---
            for j in range(NBLK):
                for k in range(NBLK):
                    nc.tensor.matmul(
                        z_ps[:, j * BSH : (j + 1) * BSH],
                        mzt_blk(k, j),
                        a_even[:, k * BSH : (k + 1) * BSH],
                        start=False,
                        stop=False,
                        skip_group_check=True,
                    )

            if not last:
                # q = Yhat + l(l-1)*Z_post
                q_t = ppool.tile([128, FREE], F32R, tag="q", name=f"q_{s}")
                nc.vector.scalar_tensor_tensor(
                    q_t[:], z_ps[:], LCOUP * (LCOUP - 1.0), yc_t[:],
                    mybir.AluOpType.mult, mybir.AluOpType.add,
                )

                # next Y-bank: bias + inv_l*q (fp32r identity), both off the
                # a_odd critical path, then the a_odd matmuls close the group
                y_next = ypool.tile([128, FREE], F32, tag="y", name=f"y_{s}")
                nc.tensor.matmul(
                    y_next[:], dby[:, s * 128 : (s + 1) * 128], indb[:],
                    start=True, stop=False,
                )
                nc.tensor.matmul(
                    y_next[:], ib32[:], q_t[:],
                    start=False, stop=False,
                )

            # --- odd eval: a_o = tanh(Z) ---
            a_odd = abuf_o[chunk][:, ecol : ecol + FREE]
            nc.scalar.activation(
                a_odd[:], z_ps[:], mybir.ActivationFunctionType.Tanh, scale=1.0
            )

            if not last:
                for j in range(NBLK):
                    for k in range(NBLK):
                        nc.tensor.matmul(
                            y_next[:, j * BSH : (j + 1) * BSH],
                            mzt_blk(k, j),
                            a_odd[:, k * BSH : (k + 1) * BSH],
                            start=False,
                            stop=(j == NBLK - 1 and k == NBLK - 1),
                        )
                y_cur = y_next

                # z bias delta for the NEXT step (after this step's z reads)
                nc.tensor.matmul(
                    z_ps[:], dbz[:, (s + 1) * 128 : (s + 2) * 128], indb[:],
                    start=False, stop=False, skip_group_check=True,
                )

            if (s + 1) % CSTEPS == 0:
                c0 = chunk * CCOLS
                nc.sync.dma_start(ae_out_d[:, c0 : c0 + CCOLS], abuf_e[chunk][:])
                nc.sync.dma_start(ao_out_d[:, c0 : c0 + CCOLS], abuf_o[chunk][:])

    nc.compile()
    return nc


_CACHE = {}


def _get_kernel():
    if "nc" not in _CACHE:
        _CACHE["nc"] = _build_kernel()
    return _CACHE["nc"]


def kernel(y1, W1, b1, u1, W2, b2, _trace=False, _trace_kwargs=None):
    y1 = np.asarray(y1)
    in_dtype = y1.dtype
    W1_ = np.asarray(W1, dtype=np.float64)
    W2_ = np.asarray(W2, dtype=np.float64)
    b2_ = np.asarray(b2, dtype=np.float64)
    tabs = _host_tables(
        np.asarray(W1), np.asarray(b1), np.asarray(u1), np.asarray(W2), np.asarray(b2)
    )

    nc = _get_kernel()

    shared = {k: tabs[k] for k in SHARED_INPUTS}
    in_maps = []
    for c in range(NCORES):
        m = dict(shared)
        r0 = c * BS
        shard = y1[r0 : r0 + BSH].astype(np.float64)  # [BSH, D]
        m["y1t0"] = np.ascontiguousarray(shard.T).astype(np.float32)
        m["y1tl0"] = np.ascontiguousarray((LCOUP * shard).T).astype(np.float32)
        in_maps.append(m)

    kw = {}
    if _trace:
        kw["trace"] = True
        if _trace_kwargs:
            kw.update(_trace_kwargs)
    res = run_bass_kernel_spmd(nc, in_maps, core_ids=list(range(NCORES)), **kw)

    # --- exact host-side output extraction ---
    gamma, c_y, c_b = _coefficients()
    cvec = np.sum(W1_ * W2_.T, axis=1)  # diag(W1@W2)
    sum_c = float(np.sum(cvec))

    out = np.zeros((B, D + 1), dtype=np.float32)
    for c in range(NCORES):
        ae = np.asarray(res.results[c]["ae_out0"]).astype(np.float64)
        ao = np.asarray(res.results[c]["ao_out0"]).astype(np.float64)
        ae = ae.reshape(128, NSTEP, NBLK, BSH)  # [p, s, blk, b]
        ao = ao.reshape(128, NSTEP, NBLK, BSH)
        ae = np.moveaxis(ae, (2, 0), (1, 2)).reshape(NSTEP, H, BSH)  # [s,h,b]
        ao = np.moveaxis(ao, (2, 0), (1, 2)).reshape(NSTEP, H, BSH)

        S = np.einsum("s,shb->hb", gamma[0::2], ae) + np.einsum(
            "s,shb->hb", gamma[1::2], ao
        )
        r0 = c * BS
        shard = y1[r0 : r0 + BSH].astype(np.float64)  # [BSH, D]
        y_fin = c_y * shard + (W2_ @ S).T + c_b * b2_[None, :]
        ptr = np.einsum("h,shb->b", cvec, ae**2)
        i_fin = HSTEP * (NSTEP * sum_c - ptr)
        out[r0 : r0 + BSH, :D] = y_fin.astype(np.float32)
        out[r0 : r0 + BSH, D] = i_fin.astype(np.float32)

    if _trace:
        return out.astype(in_dtype, copy=False), res
    return out.astype(in_dtype, copy=False)


# revision 6
# speedup vs baseline: 1.1954x; 1.1954x over previous
"""Trainium2 Bass kernel for the CNF reversible backward solve.

Math restructuring (exact, validated in fp64 against the jax reference):

The per-step recursion is tracked purely in H-space (H=256) via
Z = W1 z + bp(s), Yhat = l*(W1 y + be(s)):
    a_even = tanh(inv_l * Yhat)
    Z     += Mz @ a_even  (+ per-step bias delta)      (Mz = -h W1 W2)
    a_odd  = tanh(Z)
    Yhat'  = dby(s) + inv_l*q + Mz @ a_odd,   q = Yhat + l(l-1)*Z_post

Both states are PERMANENT in-place PSUM accumulations (Z and Yhat both
live in one PSUM bank each for the whole run). The Yhat update is written
as Yhat += dby + [eps*Yhat + (l-1)*Z] + Mz a_odd with eps = inv_l - 1 ~
1e-3: the bracketed carry correction v is ~1e-3-scaled, so it can ride
through bf16 (one DVE tensor_scalar producing eps*Yhat during the even
ACT, one DVE scalar_tensor_tensor adding (l-1)*Z post-z-update, one bf16
identity matmul injecting it) with negligible rounding: the LARGE state
never leaves fp32 PSUM. This kills the fp32 identity matmul, the Y bank
rotation, the (l-1)*Mz matmuls and the compensated hi/lo split.

The device streams all activations a_e, a_o to DRAM; the D-space outputs
are exact fp64 host-side postprocessing:
    y_final = c_y y1 + sum_e gamma_e (W2 @ a_e) + c_b b2
    I_final = h (N sum(c) - sum_s c . a_even_s^2),   c = diag(W1 W2)

Sharding: data-parallel, B=256 -> 32 samples on each of 8 cores;
parameters replicated; gather + assembly on host.
"""

import numpy as np
import ml_dtypes
from contextlib import ExitStack

import concourse.bass as bass
import concourse.tile as tile
from concourse import bacc, mybir
from concourse.bass_utils import run_bass_kernel_spmd

# Problem constants (hardcoded per contract)
NCORES = 8
B, D, H = 256, 64, 256
NSTEP = 64
HSTEP = 1.0 / NSTEP
LCOUP = 0.999
INVL = 1.0 / LCOUP
BS = B // NCORES  # 32 samples per core
BSH = BS  # samples per chain (single chain per core)
NBLK = H // 128  # 2 h-blocks
FREE = NBLK * BSH  # 64: free size of H-space tiles, layout (blk, sample)
NEVAL = 2 * NSTEP  # 128
DMA_CHUNKS = 4
CSTEPS = NSTEP // DMA_CHUNKS  # steps per out-DMA chunk
CCOLS = CSTEPS * FREE
ACOLS = NSTEP * FREE  # columns in each activation stream

F32 = mybir.dt.float32
BF16 = mybir.dt.bfloat16
BF16NP = ml_dtypes.bfloat16

SHARED_INPUTS = ["w1t", "mzt", "ib16", "dbz", "dby", "dbz0", "dby0", "ind", "indb"]


def _coefficients():
    """Exact fp64 scalar recursions for the output-extraction weights."""
    gamma = np.zeros(NEVAL)
    la = np.zeros(NEVAL)
    alpha_y = alpha_z = 1.0
    nu_y = nu_z = 0.0
    for s in range(NSTEP):
        la[2 * s] += -HSTEP
        nu_z += -HSTEP
        gamma *= INVL
        alpha_y *= INVL
        nu_y *= INVL
        gamma += (1.0 - INVL) * la
        alpha_y += (1.0 - INVL) * alpha_z
        nu_y += (1.0 - INVL) * nu_z
        gamma[2 * s + 1] += -INVL * HSTEP
        nu_y += -INVL * HSTEP
    return gamma, alpha_y, nu_y


def _host_tables(W1, b1, u1, W2, b2):
    """All precomputed tensors, fp64 internally."""
    W1 = W1.astype(np.float64)
    W2 = W2.astype(np.float64)
    b1 = b1.astype(np.float64)
    u1 = u1.astype(np.float64)
    b2 = b2.astype(np.float64)

    Mz = -HSTEP * (W1 @ W2)  # [H, H]
    W1b2 = W1 @ b2  # [H]
    l = LCOUP

    def be(s):
        return b1 + (1.0 - s * HSTEP) * u1

    def bp(s):  # beta_odd
        return b1 + (1.0 - (s + 1) * HSTEP) * u1 - (s + 1) * HSTEP * W1b2

    # mzt_pack[p, (k*NBLK+j)*128 + m] = Mz[128*j+m, 128*k+p]
    MzT = Mz.T
    mzt_pack = np.zeros((128, NBLK * NBLK * 128))
    for k in range(NBLK):
        for j in range(NBLK):
            mzt_pack[:, (k * NBLK + j) * 128 : (k * NBLK + j + 1) * 128] = MzT[
                128 * k : 128 * k + 128, 128 * j : 128 * j + 128
            ]

    # rank-2 bias tables: lhsT slice [2, 128] at cols 128*s
    dbz = np.zeros((2, NSTEP * 128))
    dby = np.zeros((2, NSTEP * 128))
    for s in range(NSTEP):
        dz = bp(s) if s == 0 else bp(s) - bp(s - 1)
        for k in range(NBLK):
            dbz[k, s * 128 : (s + 1) * 128] = dz[128 * k : 128 * k + 128]
    for s in range(NSTEP - 1):
        dh = -HSTEP * W1b2 + l * be(s + 1) - (l - 1.0) * bp(s) - be(s)
        for k in range(NBLK):
            dby[k, s * 128 : (s + 1) * 128] = dh[128 * k : 128 * k + 128]
    # col-block NSTEP-1 of dby = init bias l*be(0)
    ib = l * be(0)
    for k in range(NBLK):
        dby[k, (NSTEP - 1) * 128 : NSTEP * 128] = ib[128 * k : 128 * k + 128]

    ind = np.zeros((2, FREE))
    for k in range(NBLK):
        ind[k, k * BSH : (k + 1) * BSH] = 1.0

    dbz0 = dbz[:, 0:128].astype(np.float32)
    dby0 = dby[:, (NSTEP - 1) * 128 : NSTEP * 128].astype(np.float32)

    return dict(
        mzt=mzt_pack.astype(BF16NP),
        ib16=np.eye(128).astype(BF16NP),
        dbz=dbz.astype(BF16NP),
        dby=dby.astype(BF16NP),
        dbz0=dbz0,
        dby0=dby0,
        ind=ind.astype(np.float32),
        indb=ind.astype(BF16NP),
        w1t=W1.T.astype(np.float32),
    )


def _build_kernel():
    """Build the Bass module (same program for every core)."""
    nc = bacc.Bacc("TRN2", target_bir_lowering=False, debug=False)

    y1t_d = nc.dram_tensor("y1t0", [D, BSH], F32, kind="ExternalInput").ap()
    y1tl_d = nc.dram_tensor("y1tl0", [D, BSH], F32, kind="ExternalInput").ap()
    w1t_d = nc.dram_tensor("w1t", [D, H], F32, kind="ExternalInput").ap()
    mzt_d = nc.dram_tensor("mzt", [128, NBLK * NBLK * 128], BF16, kind="ExternalInput").ap()
    ib16_d = nc.dram_tensor("ib16", [128, 128], BF16, kind="ExternalInput").ap()
    dbz_d = nc.dram_tensor("dbz", [2, NSTEP * 128], BF16, kind="ExternalInput").ap()
    dby_d = nc.dram_tensor("dby", [2, NSTEP * 128], BF16, kind="ExternalInput").ap()
    dbz0_d = nc.dram_tensor("dbz0", [2, 128], F32, kind="ExternalInput").ap()
    dby0_d = nc.dram_tensor("dby0", [2, 128], F32, kind="ExternalInput").ap()
    ind_d = nc.dram_tensor("ind", [2, FREE], F32, kind="ExternalInput").ap()
    indb_d = nc.dram_tensor("indb", [2, FREE], BF16, kind="ExternalInput").ap()

    ae_out_d = nc.dram_tensor("ae_out0", [128, ACOLS], BF16, kind="ExternalOutput").ap()
    ao_out_d = nc.dram_tensor("ao_out0", [128, ACOLS], BF16, kind="ExternalOutput").ap()

    with tile.TileContext(nc) as tc, ExitStack() as ctx:
        consts = ctx.enter_context(tc.tile_pool(name="consts", bufs=1))
        zpool = ctx.enter_context(tc.tile_pool(name="zps", bufs=1, space="PSUM"))
        ypool = ctx.enter_context(tc.tile_pool(name="yps", bufs=1, space="PSUM"))
        ppool = ctx.enter_context(tc.tile_pool(name="ptmp", bufs=2))

        # --- prime the tanh activation table early (dep-free) ---
        warm = consts.tile([1, 8], F32, tag="warm")
        nc.vector.memset(warm[:], 0.0)
        nc.scalar.activation(warm[:], warm[:], mybir.ActivationFunctionType.Tanh)

        # --- load constants (in consumption order) ---
        def cload(name, shape, dt, dram):
            t = consts.tile(shape, dt, tag=name, name=name)
            nc.sync.dma_start(t[:], dram)
            return t

        y1t = cload("y1t0", [D, BSH], F32, y1t_d)
        y1tl = cload("y1tl0", [D, BSH], F32, y1tl_d)
        w1t = cload("w1t", [D, H], F32, w1t_d)
        dbz0 = cload("dbz0", [2, 128], F32, dbz0_d)
        dby0 = cload("dby0", [2, 128], F32, dby0_d)
        ind = cload("ind", [2, FREE], F32, ind_d)
        mzt = cload("mzt", [128, NBLK * NBLK * 128], BF16, mzt_d)
        indb = cload("indb", [2, FREE], BF16, indb_d)
        dby = cload("dby", [2, NSTEP * 128], BF16, dby_d)
        dbz = cload("dbz", [2, NSTEP * 128], BF16, dbz_d)
        ib16 = cload("ib16", [128, 128], BF16, ib16_d)

        abuf_e = [
            consts.tile([128, CCOLS], BF16, tag=f"abe{c}", name=f"abe{c}")
            for c in range(DMA_CHUNKS)
        ]
        abuf_o = [
            consts.tile([128, CCOLS], BF16, tag=f"abo{c}", name=f"abo{c}")
            for c in range(DMA_CHUNKS)
        ]

        def mzt_blk(k, j):
            base = (k * NBLK + j) * 128
            return mzt[:, base : base + 128]

        # --- init: Z = W1 y1 + bp(0);  Yhat = l W1 y1 + l be(0) ---
        z_ps = zpool.tile([128, FREE], F32, tag="z", name="z")
        for j in range(NBLK):
            nc.tensor.matmul(
                z_ps[:, j * BSH : (j + 1) * BSH],
                w1t[:, 128 * j : 128 * j + 128],
                y1t[:],
                start=(j == 0),
                stop=False,
            )
        nc.tensor.matmul(z_ps[:], dbz0[:], ind[:], start=False, stop=True)

        y_cur = ypool.tile([128, FREE], F32, tag="y", name="yinit")
        for j in range(NBLK):
            nc.tensor.matmul(
                y_cur[:, j * BSH : (j + 1) * BSH],
                w1t[:, 128 * j : 128 * j + 128],
                y1tl[:],
                start=(j == 0),
                stop=False,
            )
        nc.tensor.matmul(y_cur[:], dby0[:], ind[:], start=False, stop=True)

        for s in range(NSTEP):
            last = s == NSTEP - 1
            chunk, cstep = divmod(s, CSTEPS)
            ecol = cstep * FREE

            # --- even eval: a_e = tanh(inv_l * Yhat) ---
            a_even = abuf_e[chunk][:, ecol : ecol + FREE]
            nc.scalar.activation(
                a_even[:], y_cur[:], mybir.ActivationFunctionType.Tanh,
                scale=INVL,
            )

            if not last:
                # eps*Yhat in bf16 (DVE is otherwise idle during the even
                # ACT; also gives the v STT below its one-SBUF operand)
                yc_t = ppool.tile([128, FREE], BF16, tag="yc", name=f"yc_{s}")
                nc.vector.tensor_scalar_mul(yc_t[:], y_cur[:], INVL - 1.0)

            # --- Z += Mz @ a_even ---
            for j in range(NBLK):
                for k in range(NBLK):
                    nc.tensor.matmul(
                        z_ps[:, j * BSH : (j + 1) * BSH],
                        mzt_blk(k, j),
                        a_even[:, k * BSH : (k + 1) * BSH],
                        start=False,
                        stop=False,
                        skip_group_check=True,
                    )

            if not last:
                # v = eps*Yhat + (l-1)*Z_post in bf16 (the whole carry
                # correction is ~1e-3-scaled, so bf16 is harmless)
                v_t = ppool.tile([128, FREE], BF16, tag="v", name=f"v_{s}")
                nc.vector.scalar_tensor_tensor(
                    v_t[:], z_ps[:], LCOUP - 1.0, yc_t[:],
                    mybir.AluOpType.mult, mybir.AluOpType.add,
                )

                # Y corrections: bias + identity@v, in-queue ahead of the
                # a_odd matmuls so they execute inside the odd-ACT window
                nc.tensor.matmul(
                    y_cur[:], dby[:, s * 128 : (s + 1) * 128], indb[:],
                    start=False, stop=False, skip_group_check=True,
                )
                nc.tensor.matmul(
                    y_cur[:], ib16[:], v_t[:],
                    start=False, stop=False, skip_group_check=True,
                )

            # --- odd eval: a_o = tanh(Z) ---
            a_odd = abuf_o[chunk][:, ecol : ecol + FREE]
            nc.scalar.activation(
                a_odd[:], z_ps[:], mybir.ActivationFunctionType.Tanh, scale=1.0
            )

            if not last:
                for j in range(NBLK):
                    for k in range(NBLK):
                        nc.tensor.matmul(
                            y_cur[:, j * BSH : (j + 1) * BSH],
                            mzt_blk(k, j),
                            a_odd[:, k * BSH : (k + 1) * BSH],
                            start=False,
                            stop=False,
                            skip_group_check=True,
                        )

                # z bias delta for the NEXT step (after this step's z reads)
                nc.tensor.matmul(
                    z_ps[:], dbz[:, (s + 1) * 128 : (s + 2) * 128], indb[:],
                    start=False, stop=False, skip_group_check=True,
                )

            if (s + 1) % CSTEPS == 0:
                c0 = chunk * CCOLS
                nc.sync.dma_start(ae_out_d[:, c0 : c0 + CCOLS], abuf_e[chunk][:])
                nc.sync.dma_start(ao_out_d[:, c0 : c0 + CCOLS], abuf_o[chunk][:])

    nc.compile()
    return nc


_CACHE = {}


def _get_kernel():
    if "nc" not in _CACHE:
        _CACHE["nc"] = _build_kernel()
    return _CACHE["nc"]


def kernel(y1, W1, b1, u1, W2, b2, _trace=False, _trace_kwargs=None):
    y1 = np.asarray(y1)
    in_dtype = y1.dtype
    W1_ = np.asarray(W1, dtype=np.float64)
    W2_ = np.asarray(W2, dtype=np.float64)
    b2_ = np.asarray(b2, dtype=np.float64)
    tabs = _host_tables(
        np.asarray(W1), np.asarray(b1), np.asarray(u1), np.asarray(W2), np.asarray(b2)
    )

    nc = _get_kernel()

    shared = {k: tabs[k] for k in SHARED_INPUTS}
    in_maps = []
    for c in range(NCORES):
        m = dict(shared)
        r0 = c * BS
        shard = y1[r0 : r0 + BSH].astype(np.float64)  # [BSH, D]
        m["y1t0"] = np.ascontiguousarray(shard.T).astype(np.float32)
        m["y1tl0"] = np.ascontiguousarray((LCOUP * shard).T).astype(np.float32)
        in_maps.append(m)

    kw = {}
    if _trace:
        kw["trace"] = True
        if _trace_kwargs:
            kw.update(_trace_kwargs)
    res = run_bass_kernel_spmd(nc, in_maps, core_ids=list(range(NCORES)), **kw)

    # --- exact host-side output extraction ---
    gamma, c_y, c_b = _coefficients()
    cvec = np.sum(W1_ * W2_.T, axis=1)  # diag(W1@W2)
    sum_c = float(np.sum(cvec))

    out = np.zeros((B, D + 1), dtype=np.float32)
    for c in range(NCORES):
        ae = np.asarray(res.results[c]["ae_out0"]).astype(np.float64)
        ao = np.asarray(res.results[c]["ao_out0"]).astype(np.float64)
        ae = ae.reshape(128, NSTEP, NBLK, BSH)  # [p, s, blk, b]
        ao = ao.reshape(128, NSTEP, NBLK, BSH)
        ae = np.moveaxis(ae, (2, 0), (1, 2)).reshape(NSTEP, H, BSH)  # [s,h,b]
        ao = np.moveaxis(ao, (2, 0), (1, 2)).reshape(NSTEP, H, BSH)

        S = np.einsum("s,shb->hb", gamma[0::2], ae) + np.einsum(
            "s,shb->hb", gamma[1::2], ao
        )
        r0 = c * BS
        shard = y1[r0 : r0 + BSH].astype(np.float64)  # [BSH, D]
        y_fin = c_y * shard + (W2_ @ S).T + c_b * b2_[None, :]
        ptr = np.einsum("h,shb->b", cvec, ae**2)
        i_fin = HSTEP * (NSTEP * sum_c - ptr)
        out[r0 : r0 + BSH, :D] = y_fin.astype(np.float32)
        out[r0 : r0 + BSH, D] = i_fin.astype(np.float32)

    if _trace:
        return out.astype(in_dtype, copy=False), res
    return out.astype(in_dtype, copy=False)
